# revision 1
# baseline (speedup 1.0000x reference)
"""Distributed Trainium2 (8 NeuronCores) attention kernel.

Problem: x [8192, 256] f32; Wq/Wk/Wv [256, 256] f32 (nn.Linear layout, applied
as x @ W.T). Returns (cntx [8192, 256] f32, attn [8192, 8192] f32) where
attn = softmax((x Wq.T)(x Wk.T).T / sqrt(256)) and cntx = attn @ (x Wv.T).

Sharding: query rows are split across the 8 cores (1024 rows each); x and the
weights are replicated so each core computes K/V locally (no collectives).

Per-core pipeline (all-bf16 matmul inputs, fp32 PSUM accumulation):
  host:  xT = x.T (bf16), per-core xTq = own-row slice of xT, W*.T (bf16)
  A: QT[d, 1024] = wqT.T @ xTq           (PE, bf16)
  B: KT[d, 8192] = wkT.T @ xT, V[8192, d] (PE, bf16)
  C: per 128-row q-block:
     S = QT.T @ KT chunks -> PSUM        (PE)
     P = exp(S/16) bf16 + row-sum accum  (ACT, from PSUM, free scale)
     inv = 1/sum                         (DVE)
     P *= inv in-place                   (DVE, bf16 4x mode)
     attn out: SWDGE DMA-cast bf16->f32  (DMA)
     PT tiles = transpose(P)             (PE) -> PSUM -> SBUF copies
     cntx = PT.T @ V accumulation        (PE) -> DMA
"""

import numpy as np
import ml_dtypes

import concourse.bass as bass
import concourse.mybir as mybir
import concourse.tile as tile
from concourse import bacc
from concourse.bass_utils import run_bass_kernel_spmd
from concourse.masks import make_identity

F32 = mybir.dt.float32
BF16 = mybir.dt.bfloat16
AF = mybir.ActivationFunctionType

P = 128
N = 8192          # sequence length (rows of x)
D = 256           # d_model
NCORES = 8
QR = N // NCORES  # 1024 query rows per core
KD = D // P       # 2 k-tiles over d_model
QB = QR // P      # 8 q-blocks of 128 rows per core
CH = 1024         # score/exp chunk (2 PSUM banks)
NCHUNK = N // CH  # 8 chunks per q-block
TCH = 2048        # transpose/C chunk
SCALE = 1.0 / float(np.sqrt(D))

_CACHE = {}


def _build():
    nc = bacc.Bacc("TRN2", target_bir_lowering=False, num_devices=NCORES)

    xT = nc.dram_tensor("xT", [D, N], BF16, kind="ExternalInput")
    xTq = nc.dram_tensor("xTq", [D, QR], BF16, kind="ExternalInput")
    wqT = nc.dram_tensor("wqT", [D, D], BF16, kind="ExternalInput")
    wkT = nc.dram_tensor("wkT", [D, D], BF16, kind="ExternalInput")
    wvT = nc.dram_tensor("wvT", [D, D], BF16, kind="ExternalInput")
    attn = nc.dram_tensor("attn", [QR, N], F32, kind="ExternalOutput")
    cntx = nc.dram_tensor("cntx", [QR, D], F32, kind="ExternalOutput")

    NT = N // P  # 64 key-row tiles

    with tile.TileContext(nc) as tc:
        with (
            tc.tile_pool(name="cons", bufs=1) as cons,
            tc.tile_pool(name="sb", bufs=2) as sb,
            tc.tile_pool(name="pt", bufs=3) as ptp,
            tc.tile_pool(name="ps", bufs=2, space="PSUM") as ps,
        ):
            ident = cons.tile([P, P], BF16)
            make_identity(nc, ident)

            # ---------- load inputs ----------
            w_sb = {}
            for name, t in (("q", wqT), ("k", wkT), ("v", wvT)):
                w = cons.tile([P, KD, D], BF16, tag=f"w{name}", name=f"w_{name}")
                nc.sync.dma_start(w[:], t.rearrange("(t p) m -> p t m", p=P))
                w_sb[name] = w
            xTq_sb = cons.tile([P, KD, QR], BF16)
            nc.sync.dma_start(xTq_sb[:], xTq.rearrange("(t p) n -> p t n", p=P))
            xT_sb = cons.tile([P, KD, N], BF16)
            nc.sync.dma_start(xT_sb[:], xT.rearrange("(t p) n -> p t n", p=P))

            # ---------- A: QT[d, QR] ----------
            qt_sb = cons.tile([P, KD, QR], BF16)
            for m in range(KD):
                for c in range(QR // 512):
                    q_ps = ps.tile([P, CH], F32, tag="s", name="q_ps")[:, :512]
                    for k in range(KD):
                        nc.tensor.matmul(
                            q_ps[:], w_sb["q"][:, k, m * P:(m + 1) * P],
                            xTq_sb[:, k, c * 512:(c + 1) * 512],
                            start=(k == 0), stop=(k == KD - 1),
                        )
                    nc.scalar.copy(qt_sb[:, m, c * 512:(c + 1) * 512], q_ps[:])

            # ---------- B: KT[d, N] and V[N, d] ----------
            kt_sb = cons.tile([P, KD, N], BF16)
            for m in range(KD):
                for c in range(N // 512):
                    kt_ps = ps.tile([P, CH], F32, tag="s", name="kt_ps")[:, :512]
                    for k in range(KD):
                        nc.tensor.matmul(
                            kt_ps[:], w_sb["k"][:, k, m * P:(m + 1) * P],
                            xT_sb[:, k, c * 512:(c + 1) * 512],
                            start=(k == 0), stop=(k == KD - 1),
                        )
                    nc.scalar.copy(kt_sb[:, m, c * 512:(c + 1) * 512], kt_ps[:])

            v_sb = cons.tile([P, NT, D], BF16)
            for r in range(NT):
                v_ps = ps.tile([P, CH], F32, tag="s", name="v_ps")[:, :D]
                for k in range(KD):
                    nc.tensor.matmul(
                        v_ps[:], xT_sb[:, k, r * P:(r + 1) * P], w_sb["v"][:, k],
                        start=(k == 0), stop=(k == KD - 1),
                    )
                nc.vector.tensor_copy(v_sb[:, r], v_ps[:])

            # ---------- C: main loop over q-blocks ----------
            for qb in range(QB):
                p_sb = sb.tile([P, N], BF16, tag="p", name="p_sb")
                sums = sb.tile([P, NCHUNK], F32, tag="sums", name="sums")

                for c in range(NCHUNK):
                    s_ps = ps.tile([P, CH], F32, tag="s", name="s_ps")
                    for k in range(KD):
                        for h in range(CH // 512):
                            nc.tensor.matmul(
                                s_ps[:, h * 512:(h + 1) * 512],
                                qt_sb[:, k, qb * P:(qb + 1) * P],
                                kt_sb[:, k, c * CH + h * 512: c * CH + (h + 1) * 512],
                                start=(k == 0), stop=(k == KD - 1),
                            )
                    nc.scalar.activation(
                        p_sb[:, c * CH:(c + 1) * CH], s_ps[:], AF.Exp,
                        scale=SCALE, accum_out=sums[:, c:c + 1],
                    )

                tot = sb.tile([P, 1], F32, tag="tot", name="tot")
                nc.vector.tensor_reduce(
                    tot[:], sums[:], mybir.AxisListType.X, mybir.AluOpType.add
                )
                inv = sb.tile([P, 1], F32, tag="inv", name="inv")
                nc.vector.reciprocal(inv[:], tot[:])

                # in-place normalize (bf16 single-src -> 4x DVE mode)
                for c in range(N // TCH):
                    nc.vector.tensor_scalar_mul(
                        p_sb[:, c * TCH:(c + 1) * TCH],
                        p_sb[:, c * TCH:(c + 1) * TCH], inv[:],
                    )

                # attn shard rows: SWDGE DMA casts bf16 -> f32 on the way out
                nc.gpsimd.dma_start(attn[qb * P:(qb + 1) * P, :], p_sb[:])

                # transpose P in 128x128 tiles, accumulate cntx = PT.T @ V
                c_ps = ps.tile([P, D], F32, tag="c", name="c_ps")
                for c in range(N // TCH):
                    ptile = ptp.tile([P, TCH // P, P], BF16, tag="ptile", name="ptile")
                    for g in range(2):
                        t_ps = ps.tile([P, 8, P], BF16, tag="t", name="t_ps")
                        for j in range(8):
                            col = c * TCH + g * 8 * P + j * P
                            nc.tensor.transpose(
                                t_ps[:, j], p_sb[:, col:col + P], ident[:]
                            )
                        nc.vector.tensor_copy(ptile[:, g * 8:(g + 1) * 8], t_ps[:])
                    for j in range(TCH // P):
                        r = c * (TCH // P) + j
                        nc.tensor.matmul(
                            c_ps[:], ptile[:, j], v_sb[:, r],
                            start=(r == 0), stop=(r == NT - 1),
                        )

                cntx_sb = sb.tile([P, D], F32, tag="cntx", name="cntx_sb")
                nc.scalar.copy(cntx_sb[:], c_ps[:])
                nc.sync.dma_start(cntx[qb * P:(qb + 1) * P, :], cntx_sb[:])

    nc.compile()
    return nc


def _get_nc():
    if "nc" not in _CACHE:
        _CACHE["nc"] = _build()
    return _CACHE["nc"]


def kernel(x, Wq, Wk, Wv):
    x = np.asarray(x, dtype=np.float32)
    Wq = np.asarray(Wq, dtype=np.float32)
    Wk = np.asarray(Wk, dtype=np.float32)
    Wv = np.asarray(Wv, dtype=np.float32)

    bf = ml_dtypes.bfloat16
    xT_bf = np.ascontiguousarray(x.T).astype(bf)
    wqT_bf = np.ascontiguousarray(Wq.T).astype(bf)
    wkT_bf = np.ascontiguousarray(Wk.T).astype(bf)
    wvT_bf = np.ascontiguousarray(Wv.T).astype(bf)

    in_maps = []
    for c in range(NCORES):
        in_maps.append({
            "xT": xT_bf,
            "xTq": np.ascontiguousarray(x[c * QR:(c + 1) * QR].T).astype(bf),
            "wqT": wqT_bf,
            "wkT": wkT_bf,
            "wvT": wvT_bf,
        })

    nc = _get_nc()
    res = run_bass_kernel_spmd(nc, in_maps, list(range(NCORES)))
    attn = np.concatenate(
        [np.asarray(res.results[c]["attn"]) for c in range(NCORES)], axis=0
    )
    cntx = np.concatenate(
        [np.asarray(res.results[c]["cntx"]) for c in range(NCORES)], axis=0
    )
    return cntx, attn


# revision 4
# speedup vs baseline: 1.0124x; 1.0124x over previous
"""Distributed Trainium2 (8 NeuronCores) attention kernel.

Problem: x [8192, 256] f32; Wq/Wk/Wv [256, 256] f32 (nn.Linear layout, applied
as x @ W.T). Returns (cntx [8192, 256] f32, attn [8192, 8192] f32) where
attn = softmax((x Wq.T)(x Wk.T).T / sqrt(256)) and cntx = attn @ (x Wv.T).

Sharding: query rows are split across the 8 cores (1024 rows each); x and the
weights are replicated so each core computes K/V locally (no collectives).

Per-core pipeline (all-bf16 matmul inputs, fp32 PSUM accumulation):
  host:  xT = x.T (bf16), per-core xTq = own-row slice of xT, W*.T (bf16)
  A: QT[d, 1024] = wqT.T @ xTq           (PE, bf16)
  B: KT[d, 8192] = wkT.T @ xT, V[8192, d] (PE, bf16; chunked for pipelining)
  C: per 128-row q-block:
     S = QT.T @ KT chunks -> PSUM        (PE)
     P = exp(S/16) bf16 + row-sum accum  (ACT, from PSUM, free scale)
     inv = 1/sum                         (DVE)
     P *= inv in-place                   (DVE, bf16 4x mode)
     attn out: SWDGE DMA-cast bf16->f32  (DMA)
     PT tiles = transpose(P)             (PE) -> PSUM -> SBUF copies (DVE/ACT)
     cntx = PT.T @ V accumulation        (PE) -> DMA
"""

import numpy as np
import ml_dtypes

import concourse.bass as bass
import concourse.mybir as mybir
import concourse.tile as tile
from concourse import bacc
from concourse.bass_utils import run_bass_kernel_spmd
from concourse.masks import make_identity

F32 = mybir.dt.float32
BF16 = mybir.dt.bfloat16
AF = mybir.ActivationFunctionType

P = 128
N = 8192          # sequence length (rows of x)
D = 256           # d_model
NCORES = 8
QR = N // NCORES  # 1024 query rows per core
KD = D // P       # 2 k-tiles over d_model
QB = QR // P      # 8 q-blocks of 128 rows per core
CH = 1024         # score/exp chunk (2 PSUM banks)
NCHUNK = N // CH  # 8 chunks per q-block
XCH = 2048        # xT DMA chunk (1 MB)
TCH = 2048        # transpose/C chunk
SCALE = 1.0 / float(np.sqrt(D))

_CACHE = {}


def _build():
    nc = bacc.Bacc("TRN2", target_bir_lowering=False, num_devices=NCORES)

    xT = nc.dram_tensor("xT", [D, N], BF16, kind="ExternalInput")
    xTq = nc.dram_tensor("xTq", [D, QR], BF16, kind="ExternalInput")
    wqT = nc.dram_tensor("wqT", [D, D], BF16, kind="ExternalInput")
    wkT = nc.dram_tensor("wkT", [D, D], BF16, kind="ExternalInput")
    wvT = nc.dram_tensor("wvT", [D, D], BF16, kind="ExternalInput")
    attn = nc.dram_tensor("attn", [QR, N], F32, kind="ExternalOutput")
    cntx = nc.dram_tensor("cntx", [QR, D], F32, kind="ExternalOutput")

    NT = N // P  # 64 key-row tiles

    with tile.TileContext(nc) as tc:
        with (
            tc.tile_pool(name="cons", bufs=1) as cons,
            tc.tile_pool(name="sb", bufs=2) as sb,
            tc.tile_pool(name="pt", bufs=3) as ptp,
            tc.tile_pool(name="ps", bufs=2, space="PSUM") as ps,
            tc.tile_pool(name="pst", bufs=3, space="PSUM") as pst,
            tc.tile_pool(name="psc", bufs=1, space="PSUM") as psc,
        ):
            ident = cons.tile([P, P], BF16)
            make_identity(nc, ident)

            # ---------- load inputs (small first so projections start early) ----------
            w_sb = {}
            for name, t in (("q", wqT), ("k", wkT), ("v", wvT)):
                w = cons.tile([P, KD, D], BF16, tag=f"w{name}", name=f"w_{name}")
                nc.sync.dma_start(w[:], t.rearrange("(t p) m -> p t m", p=P))
                w_sb[name] = w
            xTq_sb = cons.tile([P, KD, QR], BF16)
            nc.sync.dma_start(xTq_sb[:], xTq.rearrange("(t p) n -> p t n", p=P))

            xt_tiles = []
            for xc in range(N // XCH):
                xt = cons.tile([P, KD, XCH], BF16, tag=f"xt{xc}", name=f"xt{xc}")
                nc.sync.dma_start(
                    xt[:],
                    xT[:, xc * XCH:(xc + 1) * XCH].rearrange("(t p) n -> p t n", p=P),
                )
                xt_tiles.append(xt)

            # ---------- A: QT[d, QR] ----------
            qt_sb = cons.tile([P, KD, QR], BF16)
            for m in range(KD):
                for c in range(QR // 512):
                    q_ps = ps.tile([P, CH], F32, tag="s", name="q_ps")[:, :512]
                    for k in range(KD):
                        nc.tensor.matmul(
                            q_ps[:], w_sb["q"][:, k, m * P:(m + 1) * P],
                            xTq_sb[:, k, c * 512:(c + 1) * 512],
                            start=(k == 0), stop=(k == KD - 1),
                        )
                    nc.scalar.copy(qt_sb[:, m, c * 512:(c + 1) * 512], q_ps[:])

            # ---------- B: KT[d, N] in CH-sized tiles (S-chunk aligned), then V ----------
            kt_tiles = []
            for c in range(NCHUNK):
                kt = cons.tile([P, KD, CH], BF16, tag=f"kt{c}", name=f"kt{c}")
                kt_tiles.append(kt)
                xt = xt_tiles[(c * CH) // XCH]
                xoff = (c * CH) % XCH
                for m in range(KD):
                    for h in range(CH // 512):
                        kt_ps = ps.tile([P, CH], F32, tag="s", name="kt_ps")[:, :512]
                        for k in range(KD):
                            nc.tensor.matmul(
                                kt_ps[:], w_sb["k"][:, k, m * P:(m + 1) * P],
                                xt[:, k, xoff + h * 512: xoff + (h + 1) * 512],
                                start=(k == 0), stop=(k == KD - 1),
                            )
                        nc.scalar.copy(
                            kt[:, m, h * 512:(h + 1) * 512], kt_ps[:]
                        )

            v_sb = cons.tile([P, NT, D], BF16)
            for r in range(NT):
                v_ps = ps.tile([P, CH], F32, tag="s", name="v_ps")[:, :D]
                xt = xt_tiles[(r * P) // XCH]
                xoff = (r * P) % XCH
                for k in range(KD):
                    nc.tensor.matmul(
                        v_ps[:], xt[:, k, xoff:xoff + P], w_sb["v"][:, k],
                        start=(k == 0), stop=(k == KD - 1),
                    )
                nc.vector.tensor_copy(v_sb[:, r], v_ps[:])

            # ---------- C: main loop over q-blocks ----------
            for qb in range(QB):
                p_sb = sb.tile([P, N], BF16, tag="p", name="p_sb")
                sums = sb.tile([P, NCHUNK], F32, tag="sums", name="sums")

                for c in range(NCHUNK):
                    s_ps = ps.tile([P, CH], F32, tag="s", name="s_ps")
                    for k in range(KD):
                        for h in range(CH // 512):
                            nc.tensor.matmul(
                                s_ps[:, h * 512:(h + 1) * 512],
                                qt_sb[:, k, qb * P:(qb + 1) * P],
                                kt_tiles[c][:, k, h * 512:(h + 1) * 512],
                                start=(k == 0), stop=(k == KD - 1),
                            )
                    nc.scalar.activation(
                        p_sb[:, c * CH:(c + 1) * CH], s_ps[:], AF.Exp,
                        scale=SCALE, accum_out=sums[:, c:c + 1],
                    )

                tot = sb.tile([P, 1], F32, tag="tot", name="tot")
                nc.vector.tensor_reduce(
                    tot[:], sums[:], mybir.AxisListType.X, mybir.AluOpType.add
                )
                inv = sb.tile([P, 1], F32, tag="inv", name="inv")
                nc.vector.reciprocal(inv[:], tot[:])

                # in-place normalize (bf16 single-src -> 4x DVE mode)
                for c in range(N // TCH):
                    nc.vector.tensor_scalar_mul(
                        p_sb[:, c * TCH:(c + 1) * TCH],
                        p_sb[:, c * TCH:(c + 1) * TCH], inv[:],
                    )

                # attn shard rows: SWDGE DMA casts bf16 -> f32 on the way out
                nc.gpsimd.dma_start(attn[qb * P:(qb + 1) * P, :], p_sb[:])

                # transpose P in 128x128 tiles, accumulate cntx = PT.T @ V
                c_ps = psc.tile([P, D], F32, tag="c", name="c_ps")
                for c in range(N // TCH):
                    ptile = ptp.tile([P, TCH // P, P], BF16, tag="ptile", name="ptile")
                    for g in range(2):
                        t_ps = pst.tile([P, 8, P], BF16, tag="t", name="t_ps")
                        for j in range(8):
                            col = c * TCH + g * 8 * P + j * P
                            nc.tensor.transpose(
                                t_ps[:, j], p_sb[:, col:col + P], ident[:]
                            )
                        eng = nc.vector if (c * 2 + g) % 2 == 0 else nc.scalar
                        if eng is nc.vector:
                            nc.vector.tensor_copy(ptile[:, g * 8:(g + 1) * 8], t_ps[:])
                        else:
                            nc.scalar.copy(ptile[:, g * 8:(g + 1) * 8], t_ps[:])
                    for j in range(TCH // P):
                        r = c * (TCH // P) + j
                        nc.tensor.matmul(
                            c_ps[:], ptile[:, j], v_sb[:, r],
                            start=(r == 0), stop=(r == NT - 1),
                        )

                cntx_sb = sb.tile([P, D], F32, tag="cntx", name="cntx_sb")
                nc.vector.tensor_copy(cntx_sb[:], c_ps[:])
                nc.sync.dma_start(cntx[qb * P:(qb + 1) * P, :], cntx_sb[:])

    nc.compile()
    return nc


def _get_nc():
    if "nc" not in _CACHE:
        _CACHE["nc"] = _build()
    return _CACHE["nc"]


def kernel(x, Wq, Wk, Wv):
    x = np.asarray(x, dtype=np.float32)
    Wq = np.asarray(Wq, dtype=np.float32)
    Wk = np.asarray(Wk, dtype=np.float32)
    Wv = np.asarray(Wv, dtype=np.float32)

    bf = ml_dtypes.bfloat16
    xT_bf = np.ascontiguousarray(x.T).astype(bf)
    wqT_bf = np.ascontiguousarray(Wq.T).astype(bf)
    wkT_bf = np.ascontiguousarray(Wk.T).astype(bf)
    wvT_bf = np.ascontiguousarray(Wv.T).astype(bf)

    in_maps = []
    for c in range(NCORES):
        in_maps.append({
            "xT": xT_bf,
            "xTq": np.ascontiguousarray(x[c * QR:(c + 1) * QR].T).astype(bf),
            "wqT": wqT_bf,
            "wkT": wkT_bf,
            "wvT": wvT_bf,
        })

    nc = _get_nc()
    res = run_bass_kernel_spmd(nc, in_maps, list(range(NCORES)))
    attn = np.concatenate(
        [np.asarray(res.results[c]["attn"]) for c in range(NCORES)], axis=0
    )
    cntx = np.concatenate(
        [np.asarray(res.results[c]["cntx"]) for c in range(NCORES)], axis=0
    )
    return cntx, attn
